# revision 1
# baseline (speedup 1.0000x reference)
"""Hybrid edge-parallel GNN kernel for 8 trn2 NeuronCores.

Baseline profiling: the two per-edge SWDGE dma_gathers (2x75136 descs
@ ~2.84ns/desc on 4 queues) cost ~427us of the 672us total. This kernel
removes the src-side dma_gather entirely:

  - Edges are sharded by SRC core, so each core's c[src] values are its
    own locally-projected nodes. They are gathered from a replicated
    SBUF table by gpsimd.ap_gather (d=16 "hex" rows, one column serves
    up to 16 edges sharing a hex), with a 16-wide DVE mask-select.
  - The dst side keeps the SWDGE dma_gather (8-wide rows in a DRAM
    p-table built after a p-only AllGather), stream-ordered to land each
    edge's p[dst] on the same (partition, column) slot as its c[src].
  - Tail |p - c + (bp-bc)|*w1 + b1 on DVE/ACT; host unpermutes.

Slot grid: (p, j) with p = 16*g + t, j in [0, W_S). ap_gather group g
column j carries one hex index; its <=16 edges sit at partitions 16g+t.
The dst dma_gather stream position j*128 + p writes slot (p, j).
"""

import numpy as np

import concourse.bacc as bacc
import concourse.tile as tile
from concourse import bass, mybir
from concourse import bass_utils
from concourse.masks import make_identity

N_CORES = 8
N_NODES = 100000
N_EDGES = 600000
IN_CH = 128

NPC = 12500          # real nodes per core
NPC_PAD = 12544      # padded node slots per core (98*128)
T_TILES = 98
HEX = 16             # src-side entries per ap_gather row
N_HEX = NPC_PAD // HEX        # 784 rows in the local c-table
W_S = 640            # slot columns (per gpsimd group)
N_SLOTS = 128 * W_S  # 77824 slots >= edges per core (~75000)
DST_ROWS = N_CORES * NPC_PAD // 8   # 12544 8-wide rows in the p-table
NI_CHUNK = 5120                     # 16 chunks -> 4 per SWDGE queue
N_CHUNKS = N_SLOTS // NI_CHUNK      # 16
IDX_COLS = N_SLOTS // 16            # 5120 wrapped dst idx cols

F32 = mybir.dt.float32
I16 = mybir.dt.int16

_CACHED_NC = None


def _wrap16(stream):
    """dma_gather idx layout: [j%16, j//16], replicated to 8 core groups."""
    w = stream.reshape(-1, 16).T
    return np.tile(w, (8, 1))


def _wrap16_groups(q8):
    """ap_gather idx layout: group g's stream lives in its 16 partitions."""
    out = np.zeros((128, q8.shape[1] // 16), np.int16)
    for g in range(8):
        out[16 * g:16 * g + 16, :] = q8[g].reshape(-1, 16).T
    return out


def _build_nc(unroll=1, variant="full", sp=False):
    nc = bacc.Bacc("TRN2", target_bir_lowering=False, debug=False,
                   num_devices=N_CORES, num_swdge_queues=4)

    xs = nc.dram_tensor("xs", [NPC_PAD, IN_CH], F32, kind="ExternalInput")
    qd = nc.dram_tensor("qd", [128, IDX_COLS], I16, kind="ExternalInput")
    qs = nc.dram_tensor("qs", [128, W_S // 16], I16, kind="ExternalInput")
    offd = nc.dram_tensor("offd", [128, W_S], F32, kind="ExternalInput")
    offs = nc.dram_tensor("offs", [128, W_S], F32, kind="ExternalInput")
    w = nc.dram_tensor("w", [IN_CH, 2], F32, kind="ExternalInput")
    scal = nc.dram_tensor("scal", [128, 24], F32, kind="ExternalInput")
    out = nc.dram_tensor("out", [128, W_S], F32, kind="ExternalOutput")

    with tile.TileContext(nc) as tc:
        with (
            tc.tile_pool(name="cst", bufs=1) as cst,
            tc.tile_pool(name="sb", bufs=3) as sb,
            tc.tile_pool(name="t8b", bufs=1) as t8b,
            tc.tile_pool(name="gat", bufs=2) as gat,
            tc.tile_pool(name="apo", bufs=2) as apo,
            tc.tile_pool(name="ps", bufs=2, space="PSUM") as ps,
            tc.tile_pool(name="pcps", bufs=1, space="PSUM") as pcps,
            tc.tile_pool(name="dram", bufs=1, space="DRAM") as dram,
        ):
            ident = cst.tile([128, 128], F32)
            make_identity(nc, ident[:])
            w_sb = cst.tile([IN_CH, 2], F32)
            nc.sync.dma_start(out=w_sb[:], in_=w[:])
            scal_sb = cst.tile([128, 24], F32)
            nc.sync.dma_start(out=scal_sb[:], in_=scal[:])
            qd_sb = cst.tile([128, IDX_COLS], I16)
            nc.sync.dma_start(out=qd_sb[:], in_=qd[:])
            qs_sb = cst.tile([128, W_S // 16], I16)
            nc.sync.dma_start(out=qs_sb[:], in_=qs[:])
            offd_sb = cst.tile([128, W_S], F32)
            nc.sync.dma_start(out=offd_sb[:], in_=offd[:])
            offs_sb = cst.tile([128, W_S], F32)
            nc.sync.dma_start(out=offs_sb[:], in_=offs[:])

            def body():
                # ---- phase 1: project local nodes: pc[n, 0:2] = x[n] @ [Wp|Wc]
                J = 7
                G = T_TILES // J
                xs_r = xs.rearrange("(g j p) c -> g p j c", j=J, p=128)
                pc_ps = pcps.tile([128, 2 * T_TILES], F32, tag="pcps")
                for g in range(G):
                    xt = sb.tile([128, J, IN_CH], F32, tag="xt")
                    nc.sync.dma_start(out=xt[:], in_=xs_r[g])
                    for j in range(J):
                        t = g * J + j
                        tp = ps.tile([128, 128], F32, tag="tp")
                        nc.tensor.transpose(tp[:], xt[:, j, :], ident[:])
                        x_t = sb.tile([128, 128], F32, tag="x_t")
                        nc.vector.tensor_copy(x_t[:], tp[:])
                        nc.tensor.matmul(
                            out=pc_ps[:, 2 * t:2 * t + 2],
                            lhsT=x_t[:],
                            rhs=w_sb[:],
                            start=True,
                            stop=True,
                        )
                pc_sb = cst.tile([128, 2 * T_TILES], F32, tag="pcsb")
                nc.vector.tensor_copy(pc_sb[:], pc_ps[:])

                # ---- phase 2: transpose p and c into node-contiguous rows.
                # p first: it gates the AllGather -> t8 -> dst-gather long pole.
                bounce_p = dram.tile([1, NPC_PAD], F32, tag="bp")
                bounce_c = dram.tile([1, NPC_PAD], F32, tag="bc")
                for comp, bnc in ((0, bounce_p), (1, bounce_c)):
                    cp_ps = ps.tile([T_TILES, 128], F32, tag="cp")
                    nc.tensor.transpose(cp_ps[:], pc_sb[:, comp::2], ident[:])
                    row = sb.tile([T_TILES, 128], F32, tag="row")
                    nc.vector.tensor_copy(row[:], cp_ps[:])
                    nc.sync.dma_start(
                        out=bnc[0].rearrange("(t p) -> t p", p=128),
                        in_=row[:],
                    )
                    if comp == 0:
                        g_p = dram.tile([N_CORES, 1, NPC_PAD], F32, tag="gp")
                        nc.gpsimd.collective_compute(
                            "AllGather",
                            mybir.AluOpType.bypass,
                            replica_groups=[list(range(N_CORES))],
                            ins=[bounce_p.opt()],
                            outs=[g_p.opt()],
                        )

                # ---- src branch: broadcast local c row into every partition
                c_sb = cst.tile([128, NPC_PAD], F32, tag="csb")
                nc.sync.dma_start(
                    out=c_sb[:],
                    in_=bounce_c[0].rearrange("(p f) -> p f", p=1)
                    .broadcast_to([128, NPC_PAD]),
                )

                # ---- dst branch: build 8-wide 256B-row p-table
                t8 = dram.tile([DST_ROWS, 128], F32, tag="t8")
                g_sb = t8b.tile([128, N_CORES * NPC_PAD // 128], F32, tag="gsb")
                nc.sync.dma_start(
                    out=g_sb[:],
                    in_=g_p.rearrange("a one (p f) -> (a one p) f", p=16),
                )
                t8_sb = t8b.tile([128, (DST_ROWS // 128) * 64], F32, tag="t8sb")
                nc.vector.tensor_copy(
                    out=t8_sb[:].rearrange("p (r e) -> p r e", e=64)[:, :, 0:8],
                    in_=g_sb[:].rearrange("p (r e) -> p r e", e=8),
                )
                nc.sync.dma_start(
                    out=t8[:, 0:64].rearrange("(p r) e -> p r e", p=128),
                    in_=t8_sb[:].rearrange("p (r e) -> p r e", e=64),
                )

                # ---- gathers: interleave SWDGE dst chunks with Pool ap_gather
                d_val = cst.tile([128, W_S], F32, tag="dval")
                s_val = cst.tile([128, W_S], F32, tag="sval")
                iota8 = scal_sb[:, 0:8]
                iotah = scal_sb[:, 0:HEX]

                def dst_chunk(ci):
                    wdt = NI_CHUNK // 128          # slot columns per chunk
                    j0 = ci * wdt
                    gth = gat.tile([128, wdt, 64], F32, tag="gth")
                    nc.gpsimd.dma_gather(
                        out_ap=gth[:],
                        in_ap=t8[:, 0:64],
                        idxs_ap=qd_sb[:, ci * wdt * 8:(ci + 1) * wdt * 8],
                        num_idxs=NI_CHUNK,
                        num_idxs_reg=NI_CHUNK,
                        elem_size=64,
                        elem_step=128,
                        single_packet=sp,
                        queue_num=ci % 4,
                    )
                    msk = gat.tile([128, wdt, 8], F32, tag="msk")
                    nc.vector.tensor_tensor(
                        out=msk[:],
                        in0=iota8.rearrange("p (one e) -> p one e", one=1)
                        .broadcast_to([128, wdt, 8]),
                        in1=offd_sb[:, j0:j0 + wdt]
                        .rearrange("p (i one) -> p i one", one=1)
                        .broadcast_to([128, wdt, 8]),
                        op=mybir.AluOpType.is_equal,
                    )
                    nc.vector.tensor_tensor(
                        out=msk[:], in0=msk[:], in1=gth[:, :, 0:8],
                        op=mybir.AluOpType.mult,
                    )
                    nc.vector.tensor_reduce(
                        out=d_val[:, j0:j0 + wdt],
                        in_=msk[:],
                        axis=mybir.AxisListType.X,
                        op=mybir.AluOpType.add,
                    )

                def src_half(h):
                    half = W_S // 4
                    j0 = h * half
                    ap_out = apo.tile([128, half, HEX], F32, tag="apo")
                    nc.gpsimd.ap_gather(
                        out_ap=ap_out[:],
                        in_ap=c_sb[:].rearrange("p (n dd) -> p n dd", dd=HEX),
                        idxs_ap=qs_sb[:, j0 // 16:(j0 + half) // 16],
                        channels=128,
                        num_elems=N_HEX,
                        d=HEX,
                        num_idxs=half,
                    )
                    msk = apo.tile([128, half, HEX], F32, tag="smsk")
                    nc.vector.tensor_tensor(
                        out=msk[:],
                        in0=iotah.rearrange("p (one e) -> p one e", one=1)
                        .broadcast_to([128, half, HEX]),
                        in1=offs_sb[:, j0:j0 + half]
                        .rearrange("p (i one) -> p i one", one=1)
                        .broadcast_to([128, half, HEX]),
                        op=mybir.AluOpType.is_equal,
                    )
                    nc.vector.tensor_tensor(
                        out=msk[:], in0=msk[:], in1=ap_out[:],
                        op=mybir.AluOpType.mult,
                    )
                    nc.vector.tensor_reduce(
                        out=s_val[:, j0:j0 + half],
                        in_=msk[:],
                        axis=mybir.AxisListType.X,
                        op=mybir.AluOpType.add,
                    )

                if variant == "head":
                    nc.vector.memset(d_val[:], 0.0)
                    nc.vector.memset(s_val[:], 0.0)
                elif variant == "nodst":
                    nc.vector.memset(d_val[:], 0.0)
                    for h in range(4):
                        src_half(h)
                elif variant == "nosrc":
                    nc.vector.memset(s_val[:], 0.0)
                    for ci in range(N_CHUNKS):
                        dst_chunk(ci)
                else:
                    for ci in range(N_CHUNKS):
                        dst_chunk(ci)
                    for h in range(4):
                        src_half(h)

                # ---- tail: |d - s + (bp-bc)| * w1 + b1
                res = cst.tile([128, W_S], F32, tag="res")
                nc.vector.tensor_tensor(
                    out=res[:], in0=d_val[:], in1=s_val[:],
                    op=mybir.AluOpType.subtract,
                )
                nc.scalar.activation(
                    out=res[:], in_=res[:],
                    func=mybir.ActivationFunctionType.Abs,
                    bias=scal_sb[:, 16:17], scale=1.0,
                )
                nc.vector.scalar_tensor_tensor(
                    out=res[:], in0=res[:],
                    scalar=scal_sb[:, 17:18],
                    in1=scal_sb[:, 18:19].to_broadcast([128, W_S]),
                    op0=mybir.AluOpType.mult,
                    op1=mybir.AluOpType.add,
                )
                nc.sync.dma_start(out=out[:], in_=res[:])

            for _ in range(unroll):
                body()

    nc.compile()
    return nc


def _host_layout(src_k, dst_k):
    """Build slot assignment for one core's edges (already src-sharded).

    Returns qs8 [8, W_S] hex indices, offs/offd [128, W_S], qd_stream
    [N_SLOTS] rows, pos [n] flat slot index p*W_S + j.
    """
    n = len(src_k)
    lc = src_k % NPC
    q = lc // HEX
    t_off = lc % HEX

    order = np.argsort(q, kind="stable")
    qo = q[order]
    m = np.bincount(q, minlength=N_HEX)
    ncols = (m + 15) // 16                         # columns per row
    total_cols = int(ncols.sum())
    if total_cols > 8 * W_S:
        raise ValueError(f"column overflow: {total_cols} > {8 * W_S}")
    colbase = np.cumsum(ncols) - ncols             # first column id per hex
    within = np.arange(n) - np.repeat(np.cumsum(m) - m, m)
    colid = colbase[qo] + within // 16             # global column id
    t = within % 16                                # partition slot in column
    g = colid % 8
    j = colid // 8
    p = 16 * g + t

    qs8 = np.zeros((8, W_S), np.int16)
    qs8[g, j] = qo.astype(np.int16)

    offs = np.full((128, W_S), float(HEX), np.float32)
    offs[p, j] = t_off[order]

    # dst side: per-slot p-table row/offset
    d_o = dst_k[order]
    flat_p = (d_o // NPC) * NPC_PAD + (d_o % NPC)
    qd_stream = np.zeros(N_SLOTS, np.int64)
    qd_stream[j * 128 + p] = flat_p >> 3
    offd = np.full((128, W_S), 8.0, np.float32)
    offd[p, j] = (flat_p & 7).astype(np.float32)

    pos = np.empty(n, np.int64)
    pos[order] = p * W_S + j
    return qs8, offs, qd_stream, offd, pos


def kernel(x, adjs, Wp, bp, Wc, bc, W1, b1):
    global _CACHED_NC
    x = np.ascontiguousarray(np.asarray(x, dtype=np.float32))
    adjs = np.asarray(adjs)
    Wp = np.asarray(Wp, dtype=np.float32)
    bp = np.asarray(bp, dtype=np.float32)
    Wc = np.asarray(Wc, dtype=np.float32)
    bc = np.asarray(bc, dtype=np.float32)
    W1 = np.asarray(W1, dtype=np.float32)
    b1 = np.asarray(b1, dtype=np.float32)

    src = adjs[0].astype(np.int64)
    dst = adjs[1].astype(np.int64)
    core_of = src // NPC

    w = np.concatenate([Wp, Wc], axis=1)
    scal = np.zeros((128, 24), dtype=np.float32)
    scal[:, 0:16] = np.arange(16, dtype=np.float32)[None, :]
    scal[:, 16] = bp[0] - bc[0]
    scal[:, 17] = W1[0, 0]
    scal[:, 18] = b1[0]

    in_maps = []
    edge_ids = []
    positions = []
    for k in range(N_CORES):
        ek = np.nonzero(core_of == k)[0]
        edge_ids.append(ek)
        xsl = np.zeros((NPC_PAD, IN_CH), dtype=np.float32)
        xsl[:NPC] = x[k * NPC:(k + 1) * NPC]
        qs8, offs, qd_stream, offd, pos = _host_layout(src[ek], dst[ek])
        positions.append(pos)
        in_maps.append({
            "xs": xsl,
            "qd": _wrap16(qd_stream.astype(np.int16)),
            "qs": _wrap16_groups(qs8),
            "offd": offd,
            "offs": offs,
            "w": w,
            "scal": scal,
        })

    if _CACHED_NC is None:
        _CACHED_NC = _build_nc()
    res = bass_utils.run_bass_kernel_spmd(
        _CACHED_NC, in_maps, core_ids=list(range(N_CORES))
    )
    out_full = np.empty(N_EDGES, dtype=np.float32)
    for k in range(N_CORES):
        flat = res.results[k]["out"].reshape(-1)
        out_full[edge_ids[k]] = flat[positions[k]]
    return out_full



# revision 7
# speedup vs baseline: 2.9991x; 2.9991x over previous
"""Edge-parallel GNN kernel for 8 trn2 NeuronCores — fully on-chip gathers.

out[e] = |p[dst[e]] - c[src[e]] + (bp-bc)| * w1 + b1,  p = x@Wp, c = x@Wc.

Strategy (per core ks, edges sharded by src core):
  - Projection: host pre-transposes x to [128ch, 12544]; 28 one-hot
    accumulating matmuls produce psum [28, 448] = p (and c) in node-
    contiguous rows; cast fp16; bounce to DRAM.
  - AllGather the fp16 p-row (25KB) across the 8 cores.
  - c-table: broadcast DMA of the local c row to all 128 partitions.
  - p-table: group g (partitions 16g..16g+15) holds core g's p slice.
  - D-side (final grid): per group g, columns of <=16 edges sharing a
    dst-hex (dst//16 within slice g, g = dst core); gpsimd.ap_gather d=16
    fetches each column's hex row; DVE is_eq/mult/reduce mask-select picks
    each edge's p[dst] into v[128, W_F].
  - S-side: gpsimd.local_scatter directly from the c-table (7 chunks of
    1792 nodes; per-partition indices map src node -> final slot, -1
    elsewhere); 6 adds merge the chunks into s[128, W_F].
  - Tail: (v - s + (bp-bc)) -> Abs -> *w1 + b1, fp16 out; host unpermutes.

No SWDGE descriptors anywhere (the baseline spent ~476us there).
"""

import numpy as np

import concourse.bacc as bacc
import concourse.tile as tile
from concourse import bass, mybir
from concourse import bass_utils

N_CORES = 8
N_NODES = 100000
N_EDGES = 600000
IN_CH = 128
NPC = 12500
NPC_PAD = 12544          # 28 * 448, 784 hexes * 16
HEX = 16
N_HEXES = NPC_PAD // HEX  # 784
NCH = 28                 # projection chunks
CHW = 448                # nodes per chunk
W_F = 1024               # final-grid columns per group (pad; idx slices 64B-aligned)
SC_CH = 7                # local_scatter chunks
SC_W = NPC_PAD // SC_CH  # 1792 nodes per scatter chunk
DG_CH = 2                # D-side gather chunks (512 cols -> 64B-aligned idx slices)
DG_W = W_F // DG_CH      # 240 columns per chunk

F16 = mybir.dt.float16
F32 = mybir.dt.float32
I16 = mybir.dt.int16

_CACHED_NC = None


def _build_nc(unroll=1, variant="full", dbg=False):
    nc = bacc.Bacc("TRN2", target_bir_lowering=False, debug=False,
                   num_devices=N_CORES, num_swdge_queues=4)

    xt = nc.dram_tensor("xt", [128, NPC_PAD], F32, kind="ExternalInput")
    woh = nc.dram_tensor("woh", [128, 2 * NCH * NCH], F32, kind="ExternalInput")
    sidx = nc.dram_tensor("sidx", [128, NPC_PAD], I16, kind="ExternalInput")
    qd = nc.dram_tensor("qd", [128, W_F // 16], I16, kind="ExternalInput")
    offd = nc.dram_tensor("offd", [128, W_F], F16, kind="ExternalInput")
    iota = nc.dram_tensor("iota", [128, 16], F16, kind="ExternalInput")
    scal = nc.dram_tensor("scal", [128, 4], F32, kind="ExternalInput")
    out = nc.dram_tensor("out", [128, W_F], F16, kind="ExternalOutput")
    if dbg:
        out_v = nc.dram_tensor("out_v", [128, W_F], F16, kind="ExternalOutput")
        out_s = nc.dram_tensor("out_s", [128, W_F], F16, kind="ExternalOutput")
        out_pt = nc.dram_tensor("out_pt", [128, NPC_PAD], F16,
                                kind="ExternalOutput")

    with tile.TileContext(nc) as tc:
        with (
            tc.tile_pool(name="cst", bufs=1) as cst,
            tc.tile_pool(name="xb", bufs=3) as xb,
            tc.tile_pool(name="gat", bufs=2) as gat,
            tc.tile_pool(name="msk", bufs=2) as mskp,
            tc.tile_pool(name="ps", bufs=2, space="PSUM") as psp,
            tc.tile_pool(name="dram", bufs=1, space="DRAM") as dram,
        ):
            woh_sb = cst.tile([128, 2 * NCH * NCH], F32)
            nc.sync.dma_start(out=woh_sb[:], in_=woh[:])
            sidx_sb = cst.tile([128, NPC_PAD], I16)
            nc.sync.dma_start(out=sidx_sb[:], in_=sidx[:])
            qd_sb = cst.tile([128, W_F // 16], I16)
            nc.sync.dma_start(out=qd_sb[:], in_=qd[:])
            offd_sb = cst.tile([128, W_F], F16)
            nc.sync.dma_start(out=offd_sb[:], in_=offd[:])
            iota_sb = cst.tile([128, 16], F16)
            nc.sync.dma_start(out=iota_sb[:], in_=iota[:])
            scal_sb = cst.tile([128, 4], F32)
            nc.sync.dma_start(out=scal_sb[:], in_=scal[:])

            def body():
                # ---- projection: psum rows t = nodes [448t, 448t+448)
                ps_p = psp.tile([NCH, CHW], F32, tag="psp")
                ps_c = psp.tile([NCH, CHW], F32, tag="psc")
                xt_r = xt.rearrange("p (t n) -> t p n", n=CHW)
                for t in range(NCH):
                    xc = xb.tile([128, CHW], F32, tag="xc")
                    nc.sync.dma_start(out=xc[:], in_=xt_r[t])
                    nc.tensor.matmul(
                        out=ps_p[:],
                        lhsT=woh_sb[:, NCH * t:NCH * (t + 1)],
                        rhs=xc[:], start=(t == 0), stop=(t == NCH - 1))
                    nc.tensor.matmul(
                        out=ps_c[:],
                        lhsT=woh_sb[:, NCH * (NCH + t):NCH * (NCH + t + 1)],
                        rhs=xc[:], start=(t == 0), stop=(t == NCH - 1))
                pc_p = cst.tile([NCH, CHW], F16, tag="pcp")
                nc.vector.tensor_copy(out=pc_p[:], in_=ps_p[:])
                pc_c = cst.tile([NCH, CHW], F16, tag="pcc")
                nc.vector.tensor_copy(out=pc_c[:], in_=ps_c[:])

                bp = dram.tile([1, NPC_PAD], F16, tag="bp")
                nc.sync.dma_start(
                    out=bp[0].rearrange("(t n) -> t n", n=CHW), in_=pc_p[:])
                bc = dram.tile([1, NPC_PAD], F16, tag="bc")
                nc.sync.dma_start(
                    out=bc[0].rearrange("(t n) -> t n", n=CHW), in_=pc_c[:])

                g_ph = dram.tile([N_CORES, 1, NPC_PAD], F16, tag="gph")
                nc.gpsimd.collective_compute(
                    "AllGather", mybir.AluOpType.bypass,
                    replica_groups=[list(range(N_CORES))],
                    ins=[bp.opt()], outs=[g_ph.opt()])

                # ---- c table: local c row on every partition
                c_tbl = cst.tile([128, NPC_PAD], F16, tag="ctbl")
                nc.sync.dma_start(
                    out=c_tbl[:],
                    in_=bc[0].rearrange("(p f) -> p f", p=1)
                    .broadcast_to([128, NPC_PAD]))

                # ---- S side: scatter straight out of the c table
                s_val = cst.tile([128, W_F], F16, tag="sval")
                m_t = [cst.tile([128, W_F], F16, tag=f"m{k}", name=f"m{k}")
                       for k in range(SC_CH)]
                for k in range(SC_CH):
                    if variant == "nosrc":
                        nc.vector.memset(m_t[k][:], 0.0)
                        continue
                    nc.gpsimd.local_scatter(
                        out_ap=m_t[k][:],
                        data_ap=c_tbl[:, SC_W * k:SC_W * (k + 1)],
                        idxs_ap=sidx_sb[:, SC_W * k:SC_W * (k + 1)],
                        channels=128, num_elems=W_F, num_idxs=SC_W)
                with nc.allow_low_precision(reason="disjoint scatter merge"):
                    nc.vector.tensor_tensor(
                        out=m_t[0][:], in0=m_t[0][:], in1=m_t[1][:],
                        op=mybir.AluOpType.add)
                    nc.vector.tensor_tensor(
                        out=m_t[2][:], in0=m_t[2][:], in1=m_t[3][:],
                        op=mybir.AluOpType.add)
                    nc.vector.tensor_tensor(
                        out=m_t[4][:], in0=m_t[4][:], in1=m_t[5][:],
                        op=mybir.AluOpType.add)
                    nc.vector.tensor_tensor(
                        out=m_t[0][:], in0=m_t[0][:], in1=m_t[2][:],
                        op=mybir.AluOpType.add)
                    nc.vector.tensor_tensor(
                        out=m_t[4][:], in0=m_t[4][:], in1=m_t[6][:],
                        op=mybir.AluOpType.add)
                    nc.vector.tensor_tensor(
                        out=s_val[:], in0=m_t[0][:], in1=m_t[4][:],
                        op=mybir.AluOpType.add)

                # ---- p table: group g holds core g's slice
                p_tbl = cst.tile([128, NPC_PAD], F16, tag="ptbl")
                for g in range(N_CORES):
                    nc.sync.dma_start(
                        out=p_tbl[16 * g:16 * (g + 1), :],
                        in_=g_ph[g].broadcast_to([16, NPC_PAD]))

                # ---- D side: gather + mask-select into final grid
                v_val = cst.tile([128, W_F], F16, tag="vval")
                for ci in range(DG_CH):
                    j0 = ci * DG_W
                    if variant == "nodst":
                        nc.vector.memset(v_val[:, j0:j0 + DG_W], 0.0)
                        continue
                    gd = gat.tile([128, DG_W, HEX], F16, tag="gd")
                    nc.gpsimd.ap_gather(
                        out_ap=gd[:],
                        in_ap=p_tbl[:].rearrange("p (n d) -> p n d", d=HEX),
                        idxs_ap=qd_sb[:, j0 // 16:(j0 + DG_W) // 16],
                        channels=128, num_elems=N_HEXES, d=HEX,
                        num_idxs=DG_W)
                    md = mskp.tile([128, DG_W, HEX], F16, tag="md")
                    nc.vector.tensor_tensor(
                        out=md[:],
                        in0=iota_sb[:].rearrange("p (one e) -> p one e", one=1)
                        .broadcast_to([128, DG_W, HEX]),
                        in1=offd_sb[:, j0:j0 + DG_W]
                        .rearrange("p (i one) -> p i one", one=1)
                        .broadcast_to([128, DG_W, HEX]),
                        op=mybir.AluOpType.is_equal)
                    nc.vector.tensor_tensor(
                        out=md[:], in0=md[:], in1=gd[:],
                        op=mybir.AluOpType.mult)
                    with nc.allow_low_precision(reason="1-hot select sum"):
                        nc.vector.tensor_reduce(
                            out=v_val[:, j0:j0 + DG_W], in_=md[:],
                            axis=mybir.AxisListType.X, op=mybir.AluOpType.add)

                if dbg:
                    nc.sync.dma_start(out=out_v[:], in_=v_val[:])
                    nc.sync.dma_start(out=out_s[:], in_=s_val[:])
                    nc.sync.dma_start(out=out_pt[:], in_=p_tbl[:])

                # ---- tail
                res = cst.tile([128, W_F], F16, tag="res")
                with nc.allow_low_precision(reason="fp16 value path"):
                    nc.vector.scalar_tensor_tensor(
                        out=res[:], in0=v_val[:],
                        scalar=scal_sb[:, 0:1],
                        in1=s_val[:],
                        op0=mybir.AluOpType.add,
                        op1=mybir.AluOpType.subtract)
                nc.scalar.activation(
                    out=res[:], in_=res[:],
                    func=mybir.ActivationFunctionType.Abs, scale=1.0)
                with nc.allow_low_precision(reason="fp16 value path"):
                    nc.vector.scalar_tensor_tensor(
                        out=res[:], in0=res[:],
                        scalar=scal_sb[:, 1:2],
                        in1=scal_sb[:, 2:3].to_broadcast([128, W_F]),
                        op0=mybir.AluOpType.mult,
                        op1=mybir.AluOpType.add)
                nc.sync.dma_start(out=out[:], in_=res[:])

            for _ in range(unroll):
                body()

    nc.compile()
    return nc


def _host_layout(src_l, dst):
    """Per-core slot assignment. src_l local src ids, dst global dst ids.

    Returns sidx [128, NPC_PAD] i16, qd [128, W_F//16] i16,
    offd [128, W_F] f16, pos [n] (flat p * W_F + col).
    """
    n = len(src_l)
    g_of = dst // NPC
    hd = (dst % NPC) // HEX
    od = (dst % NPC) % HEX

    sidx = np.full((128, NPC_PAD), -1, np.int16)
    qd8 = np.zeros((8, W_F), np.int16)
    offd = np.full((128, W_F), float(HEX), np.float16)
    pos = np.empty(n, np.int64)

    for g in range(8):
        sel = np.nonzero(g_of == g)[0]
        if len(sel) == 0:
            continue
        order = sel[np.argsort(hd[sel], kind="stable")]
        hd_o = hd[order]
        src_use = {}          # src node -> bitmask of used t (this group)
        col = 0
        i = 0
        while i < len(order):
            j = i
            h = hd_o[i]
            while j < len(order) and hd_o[j] == h:
                j += 1
            edges = list(order[i:j])
            i = j
            while edges:
                used_t = 0
                deferred = []
                filled = 0
                for e in edges:
                    placed = False
                    if filled < 16:
                        s_node = src_l[e]
                        sm = src_use.get(s_node, 0)
                        avail = ~(used_t | sm) & 0xFFFF
                        if avail:
                            t = (avail & -avail).bit_length() - 1
                            used_t |= 1 << t
                            src_use[s_node] = sm | (1 << t)
                            filled += 1
                            p = 16 * g + t
                            sidx[p, s_node] = col
                            offd[p, col] = od[e]
                            pos[e] = p * W_F + col
                            placed = True
                    if not placed:
                        deferred.append(e)
                qd8[g, col] = h
                col += 1
                if col > W_F:
                    raise RuntimeError(f"W_F overflow in group {g}")
                edges = deferred

    qd = np.zeros((128, W_F // 16), np.int16)
    for g in range(8):
        qd[16 * g:16 * (g + 1), :] = qd8[g].reshape(-1, 16).T
    return sidx, qd, offd, pos


def kernel(x, adjs, Wp, bp, Wc, bc, W1, b1):
    global _CACHED_NC
    x = np.asarray(x, dtype=np.float32)
    adjs = np.asarray(adjs)
    Wp = np.asarray(Wp, dtype=np.float32)
    bp = np.asarray(bp, dtype=np.float32)
    Wc = np.asarray(Wc, dtype=np.float32)
    bc = np.asarray(bc, dtype=np.float32)
    W1 = np.asarray(W1, dtype=np.float32)
    b1 = np.asarray(b1, dtype=np.float32)

    src = adjs[0].astype(np.int64)
    dst = adjs[1].astype(np.int64)
    core_of = src // NPC

    woh = np.zeros((128, 2, NCH, NCH), np.float32)
    for t in range(NCH):
        woh[:, 0, t, t] = Wp[:, 0]
        woh[:, 1, t, t] = Wc[:, 0]
    woh = woh.reshape(128, 2 * NCH * NCH)

    scal = np.zeros((128, 4), np.float32)
    scal[:, 0] = bp[0] - bc[0]
    scal[:, 1] = W1[0, 0]
    scal[:, 2] = b1[0]
    iota = np.tile(np.arange(16, dtype=np.float16), (128, 1))

    in_maps = []
    edge_ids = []
    positions = []
    for k in range(N_CORES):
        ek = np.nonzero(core_of == k)[0]
        edge_ids.append(ek)
        xsl = np.zeros((128, NPC_PAD), np.float32)
        xsl[:, :NPC] = x[k * NPC:(k + 1) * NPC].T
        sidx, qd, offd, pos = _host_layout(src[ek] % NPC, dst[ek])
        positions.append(pos)
        in_maps.append({
            "xt": xsl,
            "woh": woh,
            "sidx": sidx,
            "qd": qd,
            "offd": offd,
            "iota": iota,
            "scal": scal,
        })

    if _CACHED_NC is None:
        _CACHED_NC = _build_nc()
    res = bass_utils.run_bass_kernel_spmd(
        _CACHED_NC, in_maps, core_ids=list(range(N_CORES)))
    out_full = np.empty(N_EDGES, dtype=np.float32)
    for k in range(N_CORES):
        flat = res.results[k]["out"].astype(np.float32).reshape(-1)
        out_full[edge_ids[k]] = flat[positions[k]]
    return out_full


# revision 22
# speedup vs baseline: 10.2253x; 3.4095x over previous
"""Edge-parallel GNN kernel for 8 trn2 NeuronCores — fully on-chip gathers.

out[e] = |p[dst[e]] - c[src[e]] + (bp-bc)| * w1 + b1,  p = x@Wp, c = x@Wc.

Strategy (per core ks, edges sharded by src core):
  - Projection: host pre-transposes x to [128ch, 12544]; 28 one-hot
    accumulating matmuls produce psum [28, 448] = p (and c) in node-
    contiguous rows; cast fp16; bounce to DRAM.
  - AllGather the fp16 p-row (25KB) across the 8 cores.
  - c-table: broadcast DMA of the local c row to all 128 partitions.
  - p-table: group g (partitions 16g..16g+15) holds core g's p slice.
  - D-side (final grid): per group g, columns of <=16 edges sharing a
    dst-hex (dst//16 within slice g, g = dst core); gpsimd.ap_gather d=16
    fetches each column's hex row; DVE is_eq/mult/reduce mask-select picks
    each edge's p[dst] into v[128, W_F].
  - S-side: gpsimd.local_scatter directly from the c-table (7 chunks of
    1792 nodes; per-partition indices map src node -> final slot, -1
    elsewhere); 6 adds merge the chunks into s[128, W_F].
  - Tail: (v - s + (bp-bc)) -> Abs -> *w1 + b1, fp16 out; host unpermutes.

No SWDGE descriptors anywhere (the baseline spent ~476us there).
"""

import numpy as np

import concourse.bacc as bacc
import concourse.tile as tile
from concourse import bass, mybir
from concourse import bass_utils

N_CORES = 8
N_NODES = 100000
N_EDGES = 600000
IN_CH = 128
NPC = 12500
NPC_PAD = 12544          # 28 * 448, 784 hexes * 16
HEX = 16
N_HEXES = NPC_PAD // HEX  # 784
NCH = 28                 # projection chunks
CHW = 448                # nodes per chunk
W_F = 960                # final-grid columns per group (pad; chunk starts 64B-aligned)
SC_CH = 7                # local_scatter chunks
SC_W = NPC_PAD // SC_CH  # 1792 nodes per scatter chunk
DG_CH = 2                # D-side gather chunks (512 cols -> 64B-aligned idx slices)
DG_WS = [512, 448]       # chunk widths (starts at 0 and 512 -> aligned)

F16 = mybir.dt.float16
BF16 = mybir.dt.bfloat16
F32 = mybir.dt.float32
I16 = mybir.dt.int16

_CACHED_NC = None


def _build_nc(unroll=1, variant="full", dbg=False):
    nc = bacc.Bacc("TRN2", target_bir_lowering=False, debug=False,
                   num_devices=N_CORES, num_swdge_queues=4)

    xt = nc.dram_tensor("xt", [128, NPC_PAD], BF16, kind="ExternalInput")
    woh = nc.dram_tensor("woh", [128, 2 * NCH * NCH], BF16, kind="ExternalInput")
    sidx = nc.dram_tensor("sidx", [128, NPC_PAD], I16, kind="ExternalInput")
    qd = nc.dram_tensor("qd", [128, W_F // 16], I16, kind="ExternalInput")
    offd = nc.dram_tensor("offd", [128, W_F], F16, kind="ExternalInput")
    iota = nc.dram_tensor("iota", [128, 16], F16, kind="ExternalInput")
    scal = nc.dram_tensor("scal", [128, 4], F32, kind="ExternalInput")
    out = nc.dram_tensor("out", [128, W_F], F16, kind="ExternalOutput")
    if dbg:
        out_v = nc.dram_tensor("out_v", [128, W_F], F16, kind="ExternalOutput")
        out_s = nc.dram_tensor("out_s", [128, W_F], F16, kind="ExternalOutput")
        out_pt = nc.dram_tensor("out_pt", [128, NPC_PAD], F16,
                                kind="ExternalOutput")

    with tile.TileContext(nc) as tc:
        with (
            tc.tile_pool(name="cst", bufs=1) as cst,
            tc.tile_pool(name="xb", bufs=3) as xb,
            tc.tile_pool(name="gat", bufs=1) as gat,
            tc.tile_pool(name="msk", bufs=1) as mskp,
            tc.tile_pool(name="ps", bufs=2, space="PSUM") as psp,
            tc.tile_pool(name="dram", bufs=1, space="DRAM") as dram,
        ):
            woh_sb = cst.tile([128, 2 * NCH * NCH], BF16)
            nc.sync.dma_start(out=woh_sb[:], in_=woh[:])
            sidx_sb = cst.tile([128, NPC_PAD], I16)
            nc.sync.dma_start(out=sidx_sb[:], in_=sidx[:])
            qd_sb = cst.tile([128, W_F // 16], I16)
            nc.sync.dma_start(out=qd_sb[:], in_=qd[:])
            offd_sb = cst.tile([128, W_F], F16)
            nc.sync.dma_start(out=offd_sb[:], in_=offd[:])
            iota_sb = cst.tile([128, 16], F16)
            nc.sync.dma_start(out=iota_sb[:], in_=iota[:])
            scal_sb = cst.tile([128, 4], F32)
            nc.sync.dma_start(out=scal_sb[:], in_=scal[:])

            def body():
                # ---- projection: psum rows t = p nodes, rows 28+t = c nodes
                ps_pc = psp.tile([2 * NCH, CHW], F32, tag="pspc")
                xt_r = xt.rearrange("p (t n) -> t p n", n=CHW)
                for t2 in range(NCH // 2):
                    xc = xb.tile([128, 2, CHW], BF16, tag="xc")
                    nc.sync.dma_start(
                        out=xc[:], in_=xt_r[2 * t2:2 * t2 + 2]
                        .rearrange("t p n -> p t n"))
                    for j in range(2):
                        t = 2 * t2 + j
                        nc.tensor.matmul(
                            out=ps_pc[:],
                            lhsT=woh_sb[:, 2 * NCH * t:2 * NCH * (t + 1)],
                            rhs=xc[:, j, :],
                            start=(t == 0), stop=(t == NCH - 1))
                pc_pc = cst.tile([2 * NCH, CHW], F16, tag="pcpc")
                nc.vector.tensor_copy(out=pc_pc[:], in_=ps_pc[:])
                pc_p = pc_pc[0:NCH, :]
                pc_c = pc_pc[NCH:2 * NCH, :]

                if variant == "proj":
                    nc.sync.dma_start(out=out[0:NCH, 0:CHW], in_=pc_p)
                    return
                bp = dram.tile([16, NPC_PAD], F16, tag="bp")
                nc.sync.dma_start(
                    out=bp[:].rearrange("r (t n) -> t r n", n=CHW),
                    in_=pc_p.rearrange("t n -> t () n")
                    .broadcast_to([NCH, 16, CHW]))

                g_ph = dram.tile([N_CORES, 16, NPC_PAD], F16, tag="gph")
                if variant not in ("noag", "projag_noag"):
                    nc.gpsimd.collective_compute(
                        "AllGather", mybir.AluOpType.bypass,
                        replica_groups=[list(range(N_CORES))],
                        ins=[bp.opt()], outs=[g_ph.opt()])

                # ---- c table: row 0 from SBUF, then partition doubling
                skip_c = variant.startswith("projag")
                c_tbl = cst.tile([128, NPC_PAD], F16, tag="ctbl")
                if not skip_c:
                    bc = dram.tile([1, NPC_PAD], F16, tag="bc")
                    nc.sync.dma_start(
                        out=bc[0].rearrange("(t n) -> t n", n=CHW), in_=pc_c)
                    nc.sync.dma_start(
                        out=c_tbl[:],
                        in_=bc[0].rearrange("(p f) -> p f", p=1)
                        .broadcast_to([128, NPC_PAD]))

                # ---- S side: scatter straight out of the c table
                s_val = cst.tile([128, W_F], F16, tag="sval")
                m_t = [cst.tile([128, W_F], F16, tag=f"m{k}", name=f"m{k}")
                       for k in range(SC_CH)]
                if variant in ("nosrc", "head") or variant.startswith("projag"):
                    nc.vector.memset(s_val[:], 0.0)
                else:
                    for k in range(SC_CH):
                        nc.gpsimd.local_scatter(
                            out_ap=m_t[k][:],
                            data_ap=c_tbl[:, SC_W * k:SC_W * (k + 1)],
                            idxs_ap=sidx_sb[:, SC_W * k:SC_W * (k + 1)],
                            channels=128, num_elems=W_F, num_idxs=SC_W)
                    with nc.allow_low_precision(reason="disjoint merge"):
                        nc.vector.tensor_tensor(
                            out=m_t[0][:], in0=m_t[0][:], in1=m_t[1][:],
                            op=mybir.AluOpType.add)
                        nc.vector.tensor_tensor(
                            out=m_t[2][:], in0=m_t[2][:], in1=m_t[3][:],
                            op=mybir.AluOpType.add)
                        nc.vector.tensor_tensor(
                            out=m_t[4][:], in0=m_t[4][:], in1=m_t[5][:],
                            op=mybir.AluOpType.add)
                        nc.vector.tensor_tensor(
                            out=m_t[0][:], in0=m_t[0][:], in1=m_t[2][:],
                            op=mybir.AluOpType.add)
                        nc.vector.tensor_tensor(
                            out=m_t[4][:], in0=m_t[4][:], in1=m_t[6][:],
                            op=mybir.AluOpType.add)
                        nc.vector.tensor_tensor(
                            out=s_val[:], in0=m_t[0][:], in1=m_t[4][:],
                            op=mybir.AluOpType.add)

                # ---- p table: group g holds core g's slice
                p_tbl = cst.tile([128, NPC_PAD], F16, tag="ptbl")
                if variant == "projag_nopt":
                    nc.vector.memset(p_tbl[:, 0:16], 0.0)
                elif variant in ("noag", "projag_noag"):
                    for g in range(N_CORES):
                        nc.sync.dma_start(
                            out=p_tbl[16 * g:16 * (g + 1), :], in_=bp[:])
                else:
                    nc.sync.dma_start(
                        out=p_tbl[:],
                        in_=g_ph.rearrange("g t n -> (g t) n"))

                # ---- D side: gather + mask-select into final grid
                v_val = cst.tile([128, W_F], F16, tag="vval")
                j0 = 0
                for ci, dgw in enumerate(DG_WS):
                    if variant in ("nodst", "head") or variant.startswith("projag"):
                        nc.vector.memset(v_val[:, j0:j0 + dgw], 0.0)
                        j0 += dgw
                        continue
                    gd = gat.tile([128, dgw, HEX], F16, tag=f"gd{ci}",
                                  name=f"gd{ci}")
                    nc.gpsimd.ap_gather(
                        out_ap=gd[:],
                        in_ap=p_tbl[:].rearrange("p (n d) -> p n d", d=HEX),
                        idxs_ap=qd_sb[:, j0 // 16:(j0 + dgw) // 16],
                        channels=128, num_elems=N_HEXES, d=HEX,
                        num_idxs=dgw)
                    md = mskp.tile([128, dgw, HEX], F16, tag=f"md{ci}",
                                   name=f"md{ci}")
                    nc.vector.tensor_tensor(
                        out=md[:],
                        in0=iota_sb[:].rearrange("p (one e) -> p one e", one=1)
                        .broadcast_to([128, dgw, HEX]),
                        in1=offd_sb[:, j0:j0 + dgw]
                        .rearrange("p (i one) -> p i one", one=1)
                        .broadcast_to([128, dgw, HEX]),
                        op=mybir.AluOpType.is_equal)
                    nc.vector.tensor_tensor(
                        out=md[:], in0=md[:], in1=gd[:],
                        op=mybir.AluOpType.mult)
                    with nc.allow_low_precision(reason="1-hot select sum"):
                        nc.vector.tensor_reduce(
                            out=v_val[:, j0:j0 + dgw], in_=md[:],
                            axis=mybir.AxisListType.X, op=mybir.AluOpType.add)
                    j0 += dgw

                if dbg:
                    nc.sync.dma_start(out=out_v[:], in_=v_val[:])
                    nc.sync.dma_start(out=out_s[:], in_=s_val[:])
                    nc.sync.dma_start(out=out_pt[:], in_=p_tbl[:])

                # ---- tail
                res = cst.tile([128, W_F], F16, tag="res")
                with nc.allow_low_precision(reason="fp16 value path"):
                    nc.vector.scalar_tensor_tensor(
                        out=res[:], in0=v_val[:],
                        scalar=scal_sb[:, 0:1],
                        in1=s_val[:],
                        op0=mybir.AluOpType.add,
                        op1=mybir.AluOpType.subtract)
                nc.scalar.activation(
                    out=res[:], in_=res[:],
                    func=mybir.ActivationFunctionType.Abs, scale=1.0)
                with nc.allow_low_precision(reason="fp16 value path"):
                    nc.vector.scalar_tensor_tensor(
                        out=res[:], in0=res[:],
                        scalar=scal_sb[:, 1:2],
                        in1=scal_sb[:, 2:3].to_broadcast([128, W_F]),
                        op0=mybir.AluOpType.mult,
                        op1=mybir.AluOpType.add)
                nc.sync.dma_start(out=out[:], in_=res[:])

            for _ in range(unroll):
                body()

    nc.compile()
    return nc


def _host_layout(src_l, dst):
    """Per-core slot assignment. src_l local src ids, dst global dst ids.

    Returns sidx [128, NPC_PAD] i16, qd [128, W_F//16] i16,
    offd [128, W_F] f16, pos [n] (flat p * W_F + col).
    """
    n = len(src_l)
    g_of = dst // NPC
    hd = (dst % NPC) // HEX
    od = (dst % NPC) % HEX

    sidx = np.full((128, NPC_PAD), -1, np.int16)
    qd8 = np.zeros((8, W_F), np.int16)
    offd = np.full((128, W_F), float(HEX), np.float16)
    pos = np.empty(n, np.int64)

    for g in range(8):
        sel = np.nonzero(g_of == g)[0]
        if len(sel) == 0:
            continue
        order = sel[np.argsort(hd[sel], kind="stable")]
        hd_o = hd[order]
        src_use = {}          # src node -> bitmask of used t (this group)
        col = 0
        i = 0
        while i < len(order):
            j = i
            h = hd_o[i]
            while j < len(order) and hd_o[j] == h:
                j += 1
            edges = list(order[i:j])
            i = j
            while edges:
                used_t = 0
                deferred = []
                filled = 0
                for e in edges:
                    placed = False
                    if filled < 16:
                        s_node = src_l[e]
                        sm = src_use.get(s_node, 0)
                        avail = ~(used_t | sm) & 0xFFFF
                        if avail:
                            t = (avail & -avail).bit_length() - 1
                            used_t |= 1 << t
                            src_use[s_node] = sm | (1 << t)
                            filled += 1
                            p = 16 * g + t
                            sidx[p, s_node] = col
                            offd[p, col] = od[e]
                            pos[e] = p * W_F + col
                            placed = True
                    if not placed:
                        deferred.append(e)
                qd8[g, col] = h
                col += 1
                if col > W_F:
                    raise RuntimeError(f"W_F overflow in group {g}")
                edges = deferred

    qd = np.zeros((128, W_F // 16), np.int16)
    for g in range(8):
        qd[16 * g:16 * (g + 1), :] = qd8[g].reshape(-1, 16).T
    return sidx, qd, offd, pos


def kernel(x, adjs, Wp, bp, Wc, bc, W1, b1):
    global _CACHED_NC
    x = np.asarray(x, dtype=np.float32)
    adjs = np.asarray(adjs)
    Wp = np.asarray(Wp, dtype=np.float32)
    bp = np.asarray(bp, dtype=np.float32)
    Wc = np.asarray(Wc, dtype=np.float32)
    bc = np.asarray(bc, dtype=np.float32)
    W1 = np.asarray(W1, dtype=np.float32)
    b1 = np.asarray(b1, dtype=np.float32)

    src = adjs[0].astype(np.int64)
    dst = adjs[1].astype(np.int64)
    core_of = src // NPC

    import ml_dtypes
    woh = np.zeros((128, NCH, 2 * NCH), np.float32)
    for t in range(NCH):
        woh[:, t, t] = Wp[:, 0]
        woh[:, t, NCH + t] = Wc[:, 0]
    woh = woh.reshape(128, 2 * NCH * NCH).astype(ml_dtypes.bfloat16)

    scal = np.zeros((128, 4), np.float32)
    scal[:, 0] = bp[0] - bc[0]
    scal[:, 1] = W1[0, 0]
    scal[:, 2] = b1[0]
    iota = np.tile(np.arange(16, dtype=np.float16), (128, 1))

    in_maps = []
    edge_ids = []
    positions = []
    for k in range(N_CORES):
        ek = np.nonzero(core_of == k)[0]
        edge_ids.append(ek)
        xsl = np.zeros((128, NPC_PAD), ml_dtypes.bfloat16)
        xsl[:, :NPC] = x[k * NPC:(k + 1) * NPC].T.astype(ml_dtypes.bfloat16)
        sidx, qd, offd, pos = _host_layout(src[ek] % NPC, dst[ek])
        positions.append(pos)
        in_maps.append({
            "xt": xsl,
            "woh": woh,
            "sidx": sidx,
            "qd": qd,
            "offd": offd,
            "iota": iota,
            "scal": scal,
        })

    if _CACHED_NC is None:
        _CACHED_NC = _build_nc()
    res = bass_utils.run_bass_kernel_spmd(
        _CACHED_NC, in_maps, core_ids=list(range(N_CORES)))
    out_full = np.empty(N_EDGES, dtype=np.float32)
    for k in range(N_CORES):
        flat = res.results[k]["out"].astype(np.float32).reshape(-1)
        out_full[edge_ids[k]] = flat[positions[k]]
    return out_full
